# revision 19
# baseline (speedup 1.0000x reference)
"""EnhancedGATCN Trainium2 kernel: 2-layer GAT (heads=1, edge attrs) + linear head.

Strategy (8 NeuronCores, SPMD):
  - Destination-node sharding: core k owns dst nodes [k*12544, (k+1)*12544).
  - Per layer: a 512B/row gather table T[n] = [h[n](64) | 1 | alpha_src[n] |
    alpha_dst[n]] is AllGathered across cores; each core gathers T[src[e]]
    (dma_gather, int16 idx => 4 table segments) and alpha_dst[dst[e]] from a
    local 256B/row table, computes edge softmax numerators ex[e] =
    exp(leaky_relu(as+ad+ae)), and aggregates per dst-tile with a one-hot
    matmul: psum[128 dst, 65] += B_t[c,d]^T? (lhsT=B_t) @ (ex*[h|1]).
  - Segment max is skipped (logits are O(10); exp is safe in f32 and the
    max cancels exactly in the softmax ratio).
  - W2 is applied after aggregation (linearity); the final linear layer is
    fused into the layer-2 drain in transposed layout.

Host-side prep (numpy): shard + sort edges by (dst tile pair, src segment),
pad each (tile,seg) run to a fixed RUN slots so all 8 cores share one
instruction stream (SPMD), build int16 gather indices and f32 slot arrays.

Measured (8 axon trn2 cores): Relative L2 error 4.7e-7 vs the f32 jax
reference; device execute 87-109 ms (inputs device-resident, min over
repeats).  The run is bound by dma_gather descriptor drain: random 256-512B
HBM rows drain at ~50-100 ns/row effective (per-descriptor HBM latency;
insensitive to elem size / call split / queue count / single_packet), vs
~1.4 ns/row in the instruction cost model — the two per-edge gather streams
(903k rows/core/layer incl. 13% padding) dominate everything else (PE
one-hot aggregation, DVE scalar pipeline, AllGathers) which all hide
underneath.  Note dma_gather requires single_packet=False above 1024 idxs
per call, int16 indices (hence the 4-segment table split), and the idx
tile replicated across all 8 Q7 cores ([128, n/16] = 8 copies of [16, n/16]).
"""

import numpy as np

F16 = True  # use fp16 table/one-hot/scalar pipeline

# ---- problem constants (hardcoded per task instructions) ----
N = 100_000
E = 3_200_000
IN_CH, HID, EXT = 128, 64, 3
NEG_SLOPE = 0.2
CORES = 8
NC_NODES = 12544            # 98 tiles of 128 per core
NTILES = NC_NODES // 128    # 98
NPAD = CORES * NC_NODES     # 100352
SEGS = 4
SEG_ROWS = NPAD // SEGS     # 25088  (< 32768 so int16 indices fit)
NPAIR = NTILES // 2         # 49 tile pairs per core
TBL_W = 128                 # table row width (f32) -> 512B rows
TD_W = 64                   # dst table row width -> 256B rows


def _prep(x, x_ext, edge_index, edge_weight,
          W1, att_src1, att_dst1, We1, att_e1, b1,
          W2, att_src2, att_dst2, We2, att_e2, b2,
          Wlin, blin):
    """Host prep. Returns (per_core_inputs: list[dict], consts: dict, run)."""
    x = np.asarray(x, np.float32)
    x_ext = np.asarray(x_ext, np.float32)
    src = np.asarray(edge_index[0], np.int64)
    dst = np.asarray(edge_index[1], np.int64)
    w = np.asarray(edge_weight, np.float32).reshape(-1)

    k1 = float(np.asarray(We1, np.float32).reshape(-1) @ np.asarray(att_e1, np.float32))
    k2 = float(np.asarray(We2, np.float32).reshape(-1) @ np.asarray(att_e2, np.float32))

    core = dst // NC_NODES
    tile_l = (dst % NC_NODES) // 128        # 0..97
    seg = src // SEG_ROWS                   # 0..3
    # group index within core: (tile//2)*8 + seg*2 + (tile&1)
    grp = (tile_l // 2) * 8 + seg * 2 + (tile_l & 1)
    NGRP = NPAIR * 8  # 392

    # fixed RUN per (tile, seg) across all cores
    flat = core * NGRP + grp
    counts = np.bincount(flat, minlength=CORES * NGRP)
    run = int(np.ceil(counts.max() / 128.0) * 128)
    S = NGRP * run

    xcat = np.concatenate([x, x_ext], axis=1)          # [N, 131]
    xcat_pad = np.zeros((NPAD, IN_CH + EXT), np.float32)
    xcat_pad[:N] = xcat

    order = np.argsort(flat, kind="stable")
    fs, base = flat[order], None
    # rank within group
    cum = np.zeros(CORES * NGRP + 1, np.int64)
    np.cumsum(counts, out=cum[1:])
    rank = np.arange(E, dtype=np.int64) - cum[fs]
    slot = (fs % NGRP) * run + rank                    # slot within its core
    core_o = fs // NGRP

    per_core = []
    for k in range(CORES):
        m = core_o == k
        sl = slot[m]
        e = order[m]
        src16 = np.zeros(S, np.int16)
        dst16 = np.zeros(S, np.int16)
        dstrow = np.full(S, -1.0, np.float32)
        ae1 = np.zeros(S, np.float32)
        ae2 = np.zeros(S, np.float32)
        src16[sl] = (src[e] - seg[e] * SEG_ROWS).astype(np.int16)
        dst16[sl] = (dst[e] % NC_NODES).astype(np.int16)
        dstrow[sl] = (dst[e] % 128).astype(np.float32)
        ae1[sl] = k1 * w[e]
        ae2[sl] = k2 * w[e]

        def wrap16(a):  # [128, S/16] replicated for the 8 Q7 cores
            return np.tile(a.reshape(S // 16, 16).T, (8, 1)).copy()

        def wrap128(a):  # [128, S/128], slot s -> [s%128, s//128]
            return a.reshape(S // 128, 128).T.copy()

        xcT = xcat_pad[k * NC_NODES:(k + 1) * NC_NODES].T.copy()  # [131, NC]
        hdt = np.float16 if F16 else np.float32
        per_core.append({
            "srcidx": wrap16(src16),
            "dstidx": wrap16(dst16),
            "dstrow": wrap128(dstrow).astype(hdt),
            "ae1": wrap128(ae1).astype(hdt),
            "ae2": wrap128(ae2).astype(hdt),
            "xa": np.ascontiguousarray(xcT[:IN_CH]),       # [128, NC]
            "xb": np.ascontiguousarray(xcT[IN_CH:]),       # [3, NC]
        })

    W1 = np.asarray(W1, np.float32)
    W2 = np.asarray(W2, np.float32)
    Wlin = np.asarray(Wlin, np.float32)
    consts = {
        "w1a": np.ascontiguousarray(W1[:IN_CH]),           # [128, 64]
        "w1b": np.ascontiguousarray(W1[IN_CH:]),           # [3, 64]
        "asd1": np.stack([np.asarray(att_src1, np.float32),
                          np.asarray(att_dst1, np.float32)], 1),  # [64, 2]
        "w2": W2,                                          # [64, 64] (lhsT)
        "avs2": np.tile(W2 @ np.asarray(att_src2, np.float32),
                        (128, 1)).astype(np.float16 if F16 else np.float32),
        "avd2": np.tile(W2 @ np.asarray(att_dst2, np.float32),
                        (128, 1)).astype(np.float16 if F16 else np.float32),
        "b1rep": np.tile(np.asarray(b1, np.float32), (128, 1)),   # [128, 64]
        "b2col": np.asarray(b2, np.float32).reshape(HID, 1),      # [64, 1]
        "wlina": np.ascontiguousarray(Wlin[:HID]),         # [64, 2]
        "wlinb": np.ascontiguousarray(Wlin[HID:]),         # [3, 2]
        "blincol": np.asarray(blin, np.float32).reshape(2, 1),    # [2, 1]
        "iota": np.tile(np.arange(128, dtype=np.float16 if F16 else np.float32),
                        (128, 1)),  # [128,128]
        "ident": np.eye(128, dtype=np.float32),
    }
    return per_core, consts, run


def _mirror(per_core, consts, run):
    """Numpy mirror of the device algorithm (for validation)."""
    S = 392 * run
    outs = []
    tfull = [None, None]  # layer tables
    # ---- phase A on every core: build T1 shard ----
    shards = []
    td1 = []
    for k in range(CORES):
        pc = per_core[k]
        xc = np.concatenate([pc["xa"], pc["xb"]], 0).T  # [NC, 131]
        W1 = np.concatenate([consts["w1a"], consts["w1b"]], 0)
        h = xc @ W1                                    # [NC, 64]
        asd = h @ consts["asd1"]                       # [NC, 2]
        T = np.zeros((NC_NODES, TBL_W), np.float32)
        T[:, :HID] = h
        T[:, HID] = 1.0
        T[:, HID + 1] = asd[:, 0]
        T[:, HID + 2] = asd[:, 1]
        shards.append(T)
        td1.append(asd[:, 1].copy())
    tfull[0] = np.concatenate(shards, 0)               # [NPAD, 128]

    def unwrap16(a):  # inverse of wrap16
        return a[:16].T.reshape(-1)

    def unwrap128(a):
        return a.T.reshape(-1)

    def edge_pass(k, tbl, td, ae_key, layer):
        pc = per_core[k]
        src16 = unwrap16(pc["srcidx"]).astype(np.int64)
        dstrow = unwrap128(pc["dstrow"])
        ae = unwrap128(pc[ae_key])
        num = np.zeros((NC_NODES, HID + 1), np.float32)
        # segment base per slot
        segid = (np.arange(S) // run) % 8 // 2
        gsrc = tbl[src16 + segid * SEG_ROWS]           # [S, 128]
        tof = (np.arange(S) // (8 * run)) * 2 + (np.arange(S) // run) % 2
        d16 = tof * 128 + np.maximum(dstrow, 0).astype(np.int64)
        ad = np.where(dstrow >= 0, td[d16], 0.0)       # [S]
        als = gsrc[:, HID + 1]
        z = als + ad + ae
        l = np.where(z > 0, z, NEG_SLOPE * z).astype(np.float32)
        ex = np.exp(l).astype(np.float32)
        mex = gsrc[:, :HID + 1] * ex[:, None]
        tile_of_slot = (np.arange(S) // (8 * run)) * 2 + (np.arange(S) // run) % 2
        valid = dstrow >= 0
        d_global = tile_of_slot * 128 + dstrow.astype(np.int64)
        np.add.at(num, d_global[valid], mex[valid])
        return num

    # ---- layer 1 ----
    shards2 = []
    td2 = []
    for k in range(CORES):
        num = edge_pass(k, tfull[0], td1[k], "ae1", 1)
        denp = num[:, HID] + 1e-16
        h1 = np.maximum(num[:, :HID] + consts["b1rep"][0][None, :] * denp[:, None], 0) / denp[:, None]
        a2s = h1 @ consts["avs2"][0]
        a2d = h1 @ consts["avd2"][0]
        T = np.zeros((NC_NODES, TBL_W), np.float32)
        T[:, :HID] = h1
        T[:, HID] = 1.0
        T[:, HID + 1] = a2s
        T[:, HID + 2] = a2d
        shards2.append(T)
        td2.append(a2d.copy())
    tfull[1] = np.concatenate(shards2, 0)

    # ---- layer 2 + final ----
    for k in range(CORES):
        num = edge_pass(k, tfull[1], td2[k], "ae2", 2)
        denp = num[:, HID] + 1e-16
        m = num[:, :HID] / denp[:, None]
        h2 = np.maximum(m @ consts["w2"] + consts["b2col"][:, 0][None, :], 0)
        xe = per_core[k]["xb"].T                        # [NC, 3]
        o = np.maximum(h2 @ consts["wlina"] + xe @ consts["wlinb"]
                       + consts["blincol"][:, 0][None, :], 0)
        outs.append(o)
    return np.concatenate(outs, 0)[:N]


# ============================ BASS PROGRAM ============================

_PROG_CACHE = {}


def _build_program(run, stage="full", ngroups=NPAIR, upto=9, n_devices=CORES,
                   no_ag=False):
    import concourse.bacc as bacc
    import concourse.mybir as mybir
    import concourse.tile as tile
    dt = mybir.dt
    f32 = dt.float32
    f16 = dt.float16
    fh = f16 if F16 else f32

    RUN_CH = run // 128
    C = 8 * RUN_CH            # chunks per pair-group
    GRP = 8 * run             # slots per pair-group
    S = NPAIR * GRP

    nc = bacc.Bacc("TRN2", target_bir_lowering=False, debug=False,
                   num_devices=n_devices, num_swdge_queues=4)

    def din(name, shape, d=f32):
        return nc.dram_tensor(name, shape, d, kind="ExternalInput")

    srcidx_d = din("srcidx", [128, S // 16], dt.int16)
    dstidx_d = din("dstidx", [128, S // 16], dt.int16)
    dstrow_d = din("dstrow", [128, S // 128], fh)
    ae_d = [din("ae1", [128, S // 128], fh),
            din("ae2", [128, S // 128], fh)]
    xa_d = din("xa", [128, NC_NODES])
    xb_d = din("xb", [3, NC_NODES])
    w1a_d = din("w1a", [IN_CH, HID])
    w1b_d = din("w1b", [3, HID])
    asd1_d = din("asd1", [HID, 2])
    w2_d = din("w2", [HID, HID])
    avs2_d = din("avs2", [128, HID], fh)
    avd2_d = din("avd2", [128, HID], fh)
    b1rep_d = din("b1rep", [128, HID])
    b2col_d = din("b2col", [HID, 1])
    wlina_d = din("wlina", [HID, 2])
    wlinb_d = din("wlinb", [3, 2])
    blin_d = din("blincol", [2, 1])
    iota_d = din("iota", [128, 128], fh)
    ident_d = din("ident", [128, 128])
    out_d = nc.dram_tensor("out", [2, NC_NODES], f32, kind="ExternalOutput")
    dbg_d = None
    if stage in ("phasea", "ag", "l1dump"):
        dbg_d = nc.dram_tensor("dbg", [NC_NODES, TBL_W], f32,
                               kind="ExternalOutput")

    AX = mybir.AxisListType
    OP = mybir.AluOpType
    AF = mybir.ActivationFunctionType

    with tile.TileContext(nc) as tc:
        with (
            tc.tile_pool(name="dram", bufs=1, space="DRAM") as dram,
            tc.tile_pool(name="const", bufs=1) as cpool,
            tc.tile_pool(name="persist", bufs=1) as ppool,
        ):
            town = [dram.tile([NC_NODES, TBL_W], fh, name=f"town{i}")
                    for i in range(2)]
            tfull = [dram.tile([NPAD, TBL_W], fh, name=f"tfull{i}",
                              addr_space="Shared")
                     for i in range(2)]

            iota_sb = cpool.tile([128, 128], fh)
            ident_sb = cpool.tile([128, 128], f32)
            w2_sb = cpool.tile([HID, HID], f32)
            avs2_sb = cpool.tile([128, HID], fh)
            avd2_sb = cpool.tile([128, HID], fh)
            b1rep_sb = cpool.tile([128, HID], f32)
            b2col_sb = cpool.tile([HID, 1], f32)
            wlina_sb = cpool.tile([HID, 2], f32)
            wlinb_sb = cpool.tile([3, 2], f32)
            blin_sb = cpool.tile([2, 1], f32)
            xb_sb = ppool.tile([3, NC_NODES], f32)
            for sb, d in [(iota_sb, iota_d), (ident_sb, ident_d),
                          (w2_sb, w2_d), (avs2_sb, avs2_d), (avd2_sb, avd2_d),
                          (b1rep_sb, b1rep_d), (b2col_sb, b2col_d),
                          (wlina_sb, wlina_d), (wlinb_sb, wlinb_d),
                          (blin_sb, blin_d), (xb_sb, xb_d)]:
                nc.sync.dma_start(sb[:], d[:])

            # ---------------- phase A: h1 table ----------------
            with (
                tc.tile_pool(name="pa", bufs=2) as pa,
                tc.tile_pool(name="pa_ps", bufs=2, space="PSUM") as pa_ps,
                tc.tile_pool(name="pa1", bufs=1) as pa1,
            ):
                xa_sb = pa1.tile([128, NC_NODES], f32)
                nc.sync.dma_start(xa_sb[:], xa_d[:])
                w1a_sb = pa1.tile([IN_CH, HID], f32)
                w1b_sb = pa1.tile([3, HID], f32)
                asd1_sb = pa1.tile([HID, 2], f32)
                nc.sync.dma_start(w1a_sb[:], w1a_d[:])
                nc.sync.dma_start(w1b_sb[:], w1b_d[:])
                nc.sync.dma_start(asd1_sb[:], asd1_d[:])

                hT = pa1.tile([HID, NC_NODES], f32)
                asd_own = pa1.tile([2, NC_NODES], f32)
                CK = 448
                for c in range(NC_NODES // CK):
                    sl = slice(c * CK, (c + 1) * CK)
                    ph = pa_ps.tile([HID, CK], f32, tag="ph")
                    nc.tensor.matmul(ph[:], w1a_sb[:], xa_sb[:, sl],
                                     start=True, stop=False)
                    nc.tensor.matmul(ph[:], w1b_sb[:], xb_sb[:, sl],
                                     start=False, stop=True)
                    nc.vector.tensor_copy(hT[:, sl], ph[:])
                    pa2 = pa_ps.tile([2, CK], f32, tag="pa2")
                    nc.tensor.matmul(pa2[:], asd1_sb[:], hT[:, sl],
                                     start=True, stop=True)
                    nc.vector.tensor_copy(asd_own[:, sl], pa2[:])

                for t in range(NTILES):
                    sl = slice(t * 128, (t + 1) * 128)
                    stg = pa.tile([128, 67], fh, tag="stg")
                    pt = pa_ps.tile([128, HID], f32, tag="pt")
                    nc.tensor.transpose(pt[:], hT[:, sl], ident_sb[:HID, :HID])
                    nc.vector.tensor_copy(stg[:, 0:HID], pt[:])
                    nc.vector.memset(stg[:, HID:HID + 1], 1.0)
                    pt2 = pa_ps.tile([128, 2], f32, tag="pt2")
                    nc.tensor.transpose(pt2[:], asd_own[:, sl], ident_sb[:2, :2])
                    nc.vector.tensor_copy(stg[:, HID + 1:HID + 3], pt2[:])
                    nc.sync.dma_start(town[0][sl, 0:67], stg[:])

            # ---------------- per-layer edge pass ----------------
            def edge_layer(layer, ngroups=NPAIR, upto=9):
                if no_ag:
                    pass
                else:
                    nc.gpsimd.collective_compute(
                    "AllGather", OP.bypass,
                    replica_groups=[list(range(CORES))],
                    ins=[town[layer].opt()],
                    outs=[tfull[layer].opt()],
                )
                with (
                    tc.tile_pool(name=f"eg{layer}", bufs=2) as eg,
                    tc.tile_pool(name=f"eb{layer}", bufs=3) as eb,
                    tc.tile_pool(name=f"eps{layer}", bufs=1, space="PSUM") as eps,
                    tc.tile_pool(name=f"ed{layer}", bufs=2) as ed,
                    tc.tile_pool(name=f"el{layer}", bufs=1) as el,
                ):
                    for g in range(ngroups):
                        gsl16 = slice(g * GRP // 16, (g + 1) * GRP // 16)
                        gsl128 = slice(g * C, (g + 1) * C)
                        isrc = eg.tile([128, GRP // 16], dt.int16, tag="isrc")
                        idst = eg.tile([128, GRP // 16], dt.int16, tag="idst")
                        nc.sync.dma_start(isrc[:], srcidx_d[:, gsl16])
                        nc.sync.dma_start(idst[:], dstidx_d[:, gsl16])
                        drow = eg.tile([128, C], fh, tag="drow")
                        aesb = eg.tile([128, C], fh, tag="aesb")
                        nc.sync.dma_start(drow[:], dstrow_d[:, gsl128])
                        nc.sync.dma_start(aesb[:], ae_d[layer][:, gsl128])

                        if upto < 1:
                            continue
                        gs = eg.tile([128, C * TBL_W], fh, tag="gs")
                        gs3 = gs[:].rearrange("p (c e) -> p c e", e=TBL_W)
                        gd = eg.tile([128, C * TBL_W], fh, tag="gd")
                        gd3 = gd[:].rearrange("p (c e) -> p c e", e=TBL_W)
                        for j in range(SEGS):
                            csl = slice(j * 2 * RUN_CH, (j + 1) * 2 * RUN_CH)
                            nc.gpsimd.dma_gather(
                                gs3[:, csl, :],
                                tfull[layer][j * SEG_ROWS:(j + 1) * SEG_ROWS, :],
                                isrc[:, j * 2 * run // 16:(j + 1) * 2 * run // 16],
                                2 * run, 2 * run, TBL_W, elem_step=TBL_W,
                                single_packet=False, queue_num=j)
                            if upto >= 2:
                                nc.gpsimd.dma_gather(
                                    gd3[:, csl, :], town[layer][:, :],
                                    idst[:, j * 2 * run // 16:(j + 1) * 2 * run // 16],
                                    2 * run, 2 * run, TBL_W, elem_step=TBL_W,
                                    single_packet=False, queue_num=(j + 2) % 4)

                        if upto < 3:
                            continue
                        # per-edge scalars [128, C]
                        zs = eg.tile([128, C], fh, tag="zs")
                        als = gs3[:, :, HID + 1:HID + 2].rearrange("p c e -> p (c e)")
                        ad = gd3[:, :, HID + 2:HID + 3].rearrange("p c e -> p (c e)")
                        nc.vector.tensor_tensor(zs[:], als, ad, OP.add)
                        nc.vector.tensor_tensor(zs[:], zs[:], aesb[:], OP.add)
                        lkt = eg.tile([128, C], fh, tag="lkt")
                        nc.vector.tensor_scalar(lkt[:], zs[:], NEG_SLOPE, None,
                                                OP.mult)
                        nc.vector.tensor_tensor(lkt[:], lkt[:], zs[:], OP.max)
                        exs = eg.tile([128, C], fh, tag="exs")
                        nc.scalar.activation(exs[:], lkt[:], AF.Exp)

                        if upto < 4:
                            continue
                        pts = [eps.tile([128, HID + 1], f32, tag="acc", bufs=4,
                                        name=f"acc{i}")
                               for i in range(2)]
                        for r in range(8):
                            j, i = r // 2, r % 2
                            rsl = slice(r * RUN_CH, (r + 1) * RUN_CH)
                            bt = eb.tile([128, RUN_CH * 128], fh, tag="bt")
                            bt3 = bt[:].rearrange("p (c e) -> p c e", e=128)
                            nc.vector.tensor_tensor(
                                bt3[:, :, :],
                                iota_sb[:].rearrange("p (q e) -> p q e", q=1)
                                .broadcast_to([128, RUN_CH, 128]),
                                drow[:, rsl].to_broadcast([128, RUN_CH, 128]),
                                OP.is_equal)
                            mex = eb.tile([128, RUN_CH * (HID + 1)], fh, tag="mex")
                            mex3 = mex[:].rearrange("p (c e) -> p c e", e=HID + 1)
                            nc.vector.tensor_tensor(
                                mex3[:, :, :], gs3[:, rsl, 0:HID + 1],
                                exs[:, rsl].to_broadcast([128, RUN_CH, HID + 1]),
                                OP.mult)
                            for cc in range(RUN_CH):
                                nc.tensor.matmul(
                                    pts[i][:], bt3[:, cc, :], mex3[:, cc, :],
                                    start=(j == 0 and cc == 0),
                                    stop=(j == 3 and cc == RUN_CH - 1))

                        if upto < 5:
                            continue
                        for i in range(2):
                            t = 2 * g + i
                            sl = slice(t * 128, (t + 1) * 128)
                            dsb = ed.tile([128, 1], f32, tag="dsb")
                            nc.vector.tensor_scalar(
                                dsb[:], pts[i][:, HID:HID + 1], 1e-16, None,
                                OP.add)
                            if layer == 0:
                                stg = ed.tile([128, 67], fh, tag="stg2")
                                tb = ed.tile([128, HID], f32, tag="tb")
                                nc.vector.tensor_scalar(
                                    tb[:], b1rep_sb[:], dsb[:], None, OP.mult)
                                nc.vector.tensor_tensor(
                                    tb[:], tb[:], pts[i][:, 0:HID], OP.add)
                                nc.scalar.activation(tb[:], tb[:], AF.Relu)
                                inv = ed.tile([128, 1], f32, tag="inv")
                                nc.vector.reciprocal(inv[:], dsb[:])
                                nc.vector.tensor_scalar(
                                    stg[:, 0:HID], tb[:], inv[:], None,
                                    OP.mult)
                                nc.vector.memset(stg[:, HID:HID + 1], 1.0)
                                tmp = ed.tile([128, HID], fh, tag="tmp")
                                red = ed.tile([128, 2], f32, tag="red")
                                nc.vector.tensor_tensor(
                                    tmp[:], stg[:, 0:HID], avs2_sb[:], OP.mult)
                                nc.vector.reduce_sum(
                                    red[:, 0:1], tmp[:], axis=AX.X)
                                nc.vector.tensor_tensor(
                                    tmp[:], stg[:, 0:HID], avd2_sb[:], OP.mult)
                                nc.vector.reduce_sum(
                                    red[:, 1:2], tmp[:], axis=AX.X)
                                nc.vector.tensor_copy(
                                    stg[:, HID + 1:HID + 3], red[:])
                                nc.sync.dma_start(town[1][sl, 0:67], stg[:])
                            else:
                                msb = ed.tile([128, HID], f32, tag="msb")
                                inv = ed.tile([128, 1], f32, tag="inv")
                                nc.vector.reciprocal(inv[:], dsb[:])
                                nc.vector.tensor_scalar(
                                    msb[:], pts[i][:, 0:HID], inv[:], None,
                                    OP.mult)
                                pmT = eps.tile([HID, 128], f32, tag="pmT", bufs=2)
                                nc.tensor.transpose(pmT[:], msb[:], ident_sb[:])
                                mT = ed.tile([HID, 128], f32, tag="mT")
                                nc.vector.tensor_copy(mT[:], pmT[:])
                                ph2 = eps.tile([HID, 128], f32, tag="ph2", bufs=1)
                                nc.tensor.matmul(ph2[:], w2_sb[:], mT[:],
                                                 start=True, stop=True)
                                h2T = ed.tile([HID, 128], f32, tag="h2T")
                                nc.scalar.activation(h2T[:], ph2[:], AF.Relu,
                                                     bias=b2col_sb[:])
                                po = eps.tile([2, 128], f32, tag="po", bufs=1)
                                nc.tensor.matmul(po[:], wlina_sb[:], h2T[:],
                                                 start=True, stop=False)
                                nc.tensor.matmul(po[:], wlinb_sb[:],
                                                 xb_sb[:, sl],
                                                 start=False, stop=True)
                                oT = ed.tile([2, 128], f32, tag="oT")
                                nc.scalar.activation(oT[:], po[:], AF.Relu,
                                                     bias=blin_sb[:])
                                nc.sync.dma_start(out_d[:, sl], oT[:])

            if stage == "phasea":
                with tc.tile_pool(name="dbgp", bufs=2) as dbgp:
                    for t in range(NTILES):
                        sl = slice(t * 128, (t + 1) * 128)
                        dd = dbgp.tile([128, TBL_W], f32, tag="dd")
                        nc.sync.dma_start(dd[:], town[0][sl, :])
                        nc.sync.dma_start(dbg_d[sl, :], dd[:])
            elif stage == "ag":
                nc.gpsimd.collective_compute(
                    "AllGather", OP.bypass,
                    replica_groups=[list(range(CORES))],
                    ins=[town[0].opt()],
                    outs=[tfull[0].opt()],
                )
                with tc.tile_pool(name="dbgp", bufs=2) as dbgp:
                    for t in range(NTILES):
                        sl = slice(t * 128, (t + 1) * 128)
                        dd = dbgp.tile([128, TBL_W], f32, tag="dd")
                        nc.sync.dma_start(dd[:], tfull[0][sl, :])
                        nc.sync.dma_start(dbg_d[sl, :], dd[:])
            elif stage == "layer1":
                edge_layer(0, ngroups, upto)
            elif stage == "l1dump":
                edge_layer(0, ngroups, upto)
                with tc.tile_pool(name="dbgp", bufs=2) as dbgp:
                    for t in range(NTILES):
                        sl = slice(t * 128, (t + 1) * 128)
                        dd = dbgp.tile([128, TBL_W], f32, tag="dd")
                        nc.sync.dma_start(dd[:], town[1][sl, :])
                        nc.sync.dma_start(dbg_d[sl, :], dd[:])
            else:
                edge_layer(0, ngroups, upto)
                edge_layer(1, ngroups, upto)

    nc.compile()
    return nc


def _get_program(run):
    if run not in _PROG_CACHE:
        _PROG_CACHE[run] = _build_program(run)
    return _PROG_CACHE[run]


def kernel(**inputs):
    from concourse.bass_utils import run_bass_kernel_spmd

    per_core, consts, run = _prep(**inputs)
    nc = _get_program(run)
    in_maps = [dict(consts, **pc) for pc in per_core]
    res = run_bass_kernel_spmd(nc, in_maps, core_ids=list(range(CORES)))
    out = np.concatenate([r["out"].T for r in res.results], axis=0)[:N]
    return np.ascontiguousarray(out)



# revision 21
# speedup vs baseline: 1.0285x; 1.0285x over previous
"""EnhancedGATCN Trainium2 kernel: 2-layer GAT (heads=1, edge attrs) + linear head.

Strategy (8 NeuronCores, SPMD):
  - Destination-node sharding: core k owns dst nodes [k*12544, (k+1)*12544).
  - Per layer: a 256B/row fp16 gather table T[n] = [h[n](64) | 1 |
    alpha_src[n] | alpha_dst[n] | pad] is AllGathered across cores; each
    core dma_gathers T[src[e]] from the global table (int16 idx => 4 table
    segments of 25088 rows) and T_own[dst[e]] from its local shard (for
    alpha_dst, col 66), computes edge softmax numerators ex[e] =
    exp(leaky_relu(as+ad+ae)) in fp16, and aggregates per dst-tile with a
    one-hot matmul: psum[128 dst, 65] += B_t (lhsT, fp16) @ (ex*[h|1]).
  - Segment max is skipped (logits are O(10); exp is safe and the max
    cancels exactly in the softmax ratio).
  - W2 is applied after aggregation (linearity); the final linear layer is
    fused into the layer-2 drain in transposed layout (all f32).

Host-side prep (numpy): shard + sort edges by (dst tile pair, src segment),
pad each (tile,seg) run to a fixed RUN slots so all 8 cores share one
instruction stream (SPMD), build int16 gather indices and fp16 slot arrays.

Measured (8 axon trn2 cores): Relative L2 error 1.16e-4 vs the f32 jax
reference (fp16 table/pipeline; F16=False gives 4.7e-7, ~1ms slower).
The wall-clock of one execute is dominated by a ~71-85 ms axon/PJRT
dispatch floor (a null program costs that much; it also drifts several ms
over tens of minutes, so only same-process A/B comparisons are valid).
Device-attributable time is ~4-6 ms: dma_gather streams ~1.6 ms (903k
rows/core/layer x 256B across the two streams; ~8 ns/row on one SWDGE
queue, ~2.2 ns/row spread over 4 queues via num_swdge_queues=4 +
queue_num, further helped by fp16 halving row bytes), DVE one-hot/mex
~1.4 ms, PE aggregation + drain ~1.2 ms, 2 AllGathers ~1.0 ms.
Computing alpha_dst on-chip (partition_broadcast + one-hot mult+reduce)
was tried and is SLOWER than the second gather stream (~2.1-2.6 us/run
DVE mult+reduce; fp16 gives no DVE speedup on these broadcast-AP ops).
Note dma_gather requires single_packet=False above 1024 idxs per call,
256B-multiple rows, int16 indices (hence the 4-segment table split), and
the idx tile replicated across the 8 Q7 cores ([128, n/16] = 8 copies of
[16, n/16]).
"""

import numpy as np

F16 = True  # use fp16 table/one-hot/scalar pipeline

# ---- problem constants (hardcoded per task instructions) ----
N = 100_000
E = 3_200_000
IN_CH, HID, EXT = 128, 64, 3
NEG_SLOPE = 0.2
CORES = 8
NC_NODES = 12544            # 98 tiles of 128 per core
NTILES = NC_NODES // 128    # 98
NPAD = CORES * NC_NODES     # 100352
SEGS = 4
SEG_ROWS = NPAD // SEGS     # 25088  (< 32768 so int16 indices fit)
NPAIR = NTILES // 2         # 49 tile pairs per core
TBL_W = 128                 # table row width: 128 fp16 -> 256B rows


def _prep(x, x_ext, edge_index, edge_weight,
          W1, att_src1, att_dst1, We1, att_e1, b1,
          W2, att_src2, att_dst2, We2, att_e2, b2,
          Wlin, blin):
    """Host prep. Returns (per_core_inputs: list[dict], consts: dict, run)."""
    x = np.asarray(x, np.float32)
    x_ext = np.asarray(x_ext, np.float32)
    src = np.asarray(edge_index[0], np.int64)
    dst = np.asarray(edge_index[1], np.int64)
    w = np.asarray(edge_weight, np.float32).reshape(-1)

    k1 = float(np.asarray(We1, np.float32).reshape(-1) @ np.asarray(att_e1, np.float32))
    k2 = float(np.asarray(We2, np.float32).reshape(-1) @ np.asarray(att_e2, np.float32))

    core = dst // NC_NODES
    tile_l = (dst % NC_NODES) // 128        # 0..97
    seg = src // SEG_ROWS                   # 0..3
    # group index within core: (tile//2)*8 + seg*2 + (tile&1)
    grp = (tile_l // 2) * 8 + seg * 2 + (tile_l & 1)
    NGRP = NPAIR * 8  # 392

    # fixed RUN per (tile, seg) across all cores
    flat = core * NGRP + grp
    counts = np.bincount(flat, minlength=CORES * NGRP)
    run = int(np.ceil(counts.max() / 128.0) * 128)
    S = NGRP * run

    xcat = np.concatenate([x, x_ext], axis=1)          # [N, 131]
    xcat_pad = np.zeros((NPAD, IN_CH + EXT), np.float32)
    xcat_pad[:N] = xcat

    order = np.argsort(flat, kind="stable")
    fs, base = flat[order], None
    # rank within group
    cum = np.zeros(CORES * NGRP + 1, np.int64)
    np.cumsum(counts, out=cum[1:])
    rank = np.arange(E, dtype=np.int64) - cum[fs]
    slot = (fs % NGRP) * run + rank                    # slot within its core
    core_o = fs // NGRP

    per_core = []
    for k in range(CORES):
        m = core_o == k
        sl = slot[m]
        e = order[m]
        src16 = np.zeros(S, np.int16)
        dst16 = np.zeros(S, np.int16)
        dstrow = np.full(S, -1.0, np.float32)
        ae1 = np.zeros(S, np.float32)
        ae2 = np.zeros(S, np.float32)
        src16[sl] = (src[e] - seg[e] * SEG_ROWS).astype(np.int16)
        dst16[sl] = (dst[e] % NC_NODES).astype(np.int16)
        dstrow[sl] = (dst[e] % 128).astype(np.float32)
        ae1[sl] = k1 * w[e]
        ae2[sl] = k2 * w[e]

        def wrap16(a):  # [128, S/16] replicated for the 8 Q7 cores
            return np.tile(a.reshape(S // 16, 16).T, (8, 1)).copy()

        def wrap128(a):  # [128, S/128], slot s -> [s%128, s//128]
            return a.reshape(S // 128, 128).T.copy()

        xcT = xcat_pad[k * NC_NODES:(k + 1) * NC_NODES].T.copy()  # [131, NC]
        hdt = np.float16 if F16 else np.float32
        per_core.append({
            "srcidx": wrap16(src16),
            "dstidx": wrap16(dst16),
            "dstrow": wrap128(dstrow).astype(hdt),
            "ae1": wrap128(ae1).astype(hdt),
            "ae2": wrap128(ae2).astype(hdt),
            "xa": np.ascontiguousarray(xcT[:IN_CH]),       # [128, NC]
            "xb": np.ascontiguousarray(xcT[IN_CH:]),       # [3, NC]
        })

    W1 = np.asarray(W1, np.float32)
    W2 = np.asarray(W2, np.float32)
    Wlin = np.asarray(Wlin, np.float32)
    consts = {
        "w1a": np.ascontiguousarray(W1[:IN_CH]),           # [128, 64]
        "w1b": np.ascontiguousarray(W1[IN_CH:]),           # [3, 64]
        "asd1": np.stack([np.asarray(att_src1, np.float32),
                          np.asarray(att_dst1, np.float32)], 1),  # [64, 2]
        "w2": W2,                                          # [64, 64] (lhsT)
        "avs2": np.tile(W2 @ np.asarray(att_src2, np.float32),
                        (128, 1)).astype(np.float16 if F16 else np.float32),
        "avd2": np.tile(W2 @ np.asarray(att_dst2, np.float32),
                        (128, 1)).astype(np.float16 if F16 else np.float32),
        "b1rep": np.tile(np.asarray(b1, np.float32), (128, 1)),   # [128, 64]
        "b2col": np.asarray(b2, np.float32).reshape(HID, 1),      # [64, 1]
        "wlina": np.ascontiguousarray(Wlin[:HID]),         # [64, 2]
        "wlinb": np.ascontiguousarray(Wlin[HID:]),         # [3, 2]
        "blincol": np.asarray(blin, np.float32).reshape(2, 1),    # [2, 1]
        "iota": np.tile(np.arange(128, dtype=np.float16 if F16 else np.float32),
                        (128, 1)),  # [128,128]
        "ident": np.eye(128, dtype=np.float32),
    }
    return per_core, consts, run


def _mirror(per_core, consts, run):
    """Numpy mirror of the device algorithm (for validation)."""
    S = 392 * run
    outs = []
    tfull = [None, None]  # layer tables
    # ---- phase A on every core: build T1 shard ----
    shards = []
    td1 = []
    for k in range(CORES):
        pc = per_core[k]
        xc = np.concatenate([pc["xa"], pc["xb"]], 0).T  # [NC, 131]
        W1 = np.concatenate([consts["w1a"], consts["w1b"]], 0)
        h = xc @ W1                                    # [NC, 64]
        asd = h @ consts["asd1"]                       # [NC, 2]
        T = np.zeros((NC_NODES, TBL_W), np.float32)
        T[:, :HID] = h
        T[:, HID] = 1.0
        T[:, HID + 1] = asd[:, 0]
        T[:, HID + 2] = asd[:, 1]
        shards.append(T)
        td1.append(asd[:, 1].copy())
    tfull[0] = np.concatenate(shards, 0)               # [NPAD, 128]

    def unwrap16(a):  # inverse of wrap16
        return a[:16].T.reshape(-1)

    def unwrap128(a):
        return a.T.reshape(-1)

    def edge_pass(k, tbl, td, ae_key, layer):
        pc = per_core[k]
        src16 = unwrap16(pc["srcidx"]).astype(np.int64)
        dstrow = unwrap128(pc["dstrow"])
        ae = unwrap128(pc[ae_key])
        num = np.zeros((NC_NODES, HID + 1), np.float32)
        # segment base per slot
        segid = (np.arange(S) // run) % 8 // 2
        gsrc = tbl[src16 + segid * SEG_ROWS]           # [S, 128]
        tof = (np.arange(S) // (8 * run)) * 2 + (np.arange(S) // run) % 2
        d16 = tof * 128 + np.maximum(dstrow, 0).astype(np.int64)
        ad = np.where(dstrow >= 0, td[d16], 0.0)       # [S]
        als = gsrc[:, HID + 1]
        z = als + ad + ae
        l = np.where(z > 0, z, NEG_SLOPE * z).astype(np.float32)
        ex = np.exp(l).astype(np.float32)
        mex = gsrc[:, :HID + 1] * ex[:, None]
        tile_of_slot = (np.arange(S) // (8 * run)) * 2 + (np.arange(S) // run) % 2
        valid = dstrow >= 0
        d_global = tile_of_slot * 128 + dstrow.astype(np.int64)
        np.add.at(num, d_global[valid], mex[valid])
        return num

    # ---- layer 1 ----
    shards2 = []
    td2 = []
    for k in range(CORES):
        num = edge_pass(k, tfull[0], td1[k], "ae1", 1)
        denp = num[:, HID] + 1e-16
        h1 = np.maximum(num[:, :HID] + consts["b1rep"][0][None, :] * denp[:, None], 0) / denp[:, None]
        a2s = h1 @ consts["avs2"][0]
        a2d = h1 @ consts["avd2"][0]
        T = np.zeros((NC_NODES, TBL_W), np.float32)
        T[:, :HID] = h1
        T[:, HID] = 1.0
        T[:, HID + 1] = a2s
        T[:, HID + 2] = a2d
        shards2.append(T)
        td2.append(a2d.copy())
    tfull[1] = np.concatenate(shards2, 0)

    # ---- layer 2 + final ----
    for k in range(CORES):
        num = edge_pass(k, tfull[1], td2[k], "ae2", 2)
        denp = num[:, HID] + 1e-16
        m = num[:, :HID] / denp[:, None]
        h2 = np.maximum(m @ consts["w2"] + consts["b2col"][:, 0][None, :], 0)
        xe = per_core[k]["xb"].T                        # [NC, 3]
        o = np.maximum(h2 @ consts["wlina"] + xe @ consts["wlinb"]
                       + consts["blincol"][:, 0][None, :], 0)
        outs.append(o)
    return np.concatenate(outs, 0)[:N]


# ============================ BASS PROGRAM ============================

_PROG_CACHE = {}


def _build_program(run, stage="full", ngroups=NPAIR, upto=9, n_devices=CORES,
                   no_ag=False):
    import concourse.bacc as bacc
    import concourse.mybir as mybir
    import concourse.tile as tile
    dt = mybir.dt
    f32 = dt.float32
    f16 = dt.float16
    fh = f16 if F16 else f32

    RUN_CH = run // 128
    C = 8 * RUN_CH            # chunks per pair-group
    GRP = 8 * run             # slots per pair-group
    S = NPAIR * GRP

    nc = bacc.Bacc("TRN2", target_bir_lowering=False, debug=False,
                   num_devices=n_devices, num_swdge_queues=4)

    def din(name, shape, d=f32):
        return nc.dram_tensor(name, shape, d, kind="ExternalInput")

    srcidx_d = din("srcidx", [128, S // 16], dt.int16)
    dstidx_d = din("dstidx", [128, S // 16], dt.int16)
    dstrow_d = din("dstrow", [128, S // 128], fh)
    ae_d = [din("ae1", [128, S // 128], fh),
            din("ae2", [128, S // 128], fh)]
    xa_d = din("xa", [128, NC_NODES])
    xb_d = din("xb", [3, NC_NODES])
    w1a_d = din("w1a", [IN_CH, HID])
    w1b_d = din("w1b", [3, HID])
    asd1_d = din("asd1", [HID, 2])
    w2_d = din("w2", [HID, HID])
    avs2_d = din("avs2", [128, HID], fh)
    avd2_d = din("avd2", [128, HID], fh)
    b1rep_d = din("b1rep", [128, HID])
    b2col_d = din("b2col", [HID, 1])
    wlina_d = din("wlina", [HID, 2])
    wlinb_d = din("wlinb", [3, 2])
    blin_d = din("blincol", [2, 1])
    iota_d = din("iota", [128, 128], fh)
    ident_d = din("ident", [128, 128])
    out_d = nc.dram_tensor("out", [2, NC_NODES], f32, kind="ExternalOutput")
    dbg_d = None
    if stage in ("phasea", "ag", "l1dump"):
        dbg_d = nc.dram_tensor("dbg", [NC_NODES, TBL_W], f32,
                               kind="ExternalOutput")

    AX = mybir.AxisListType
    OP = mybir.AluOpType
    AF = mybir.ActivationFunctionType

    with tile.TileContext(nc) as tc:
        with (
            tc.tile_pool(name="dram", bufs=1, space="DRAM") as dram,
            tc.tile_pool(name="const", bufs=1) as cpool,
            tc.tile_pool(name="persist", bufs=1) as ppool,
        ):
            town = [dram.tile([NC_NODES, TBL_W], fh, name=f"town{i}")
                    for i in range(2)]
            tfull = [dram.tile([NPAD, TBL_W], fh, name=f"tfull{i}",
                              addr_space="Shared")
                     for i in range(2)]

            iota_sb = cpool.tile([128, 128], fh)
            ident_sb = cpool.tile([128, 128], f32)
            w2_sb = cpool.tile([HID, HID], f32)
            avs2_sb = cpool.tile([128, HID], fh)
            avd2_sb = cpool.tile([128, HID], fh)
            b1rep_sb = cpool.tile([128, HID], f32)
            b2col_sb = cpool.tile([HID, 1], f32)
            wlina_sb = cpool.tile([HID, 2], f32)
            wlinb_sb = cpool.tile([3, 2], f32)
            blin_sb = cpool.tile([2, 1], f32)
            xb_sb = ppool.tile([3, NC_NODES], f32)
            for sb, d in [(iota_sb, iota_d), (ident_sb, ident_d),
                          (w2_sb, w2_d), (avs2_sb, avs2_d), (avd2_sb, avd2_d),
                          (b1rep_sb, b1rep_d), (b2col_sb, b2col_d),
                          (wlina_sb, wlina_d), (wlinb_sb, wlinb_d),
                          (blin_sb, blin_d), (xb_sb, xb_d)]:
                nc.sync.dma_start(sb[:], d[:])

            # ---------------- phase A: h1 table ----------------
            with (
                tc.tile_pool(name="pa", bufs=2) as pa,
                tc.tile_pool(name="pa_ps", bufs=2, space="PSUM") as pa_ps,
                tc.tile_pool(name="pa1", bufs=1) as pa1,
            ):
                xa_sb = pa1.tile([128, NC_NODES], f32)
                nc.sync.dma_start(xa_sb[:], xa_d[:])
                w1a_sb = pa1.tile([IN_CH, HID], f32)
                w1b_sb = pa1.tile([3, HID], f32)
                asd1_sb = pa1.tile([HID, 2], f32)
                nc.sync.dma_start(w1a_sb[:], w1a_d[:])
                nc.sync.dma_start(w1b_sb[:], w1b_d[:])
                nc.sync.dma_start(asd1_sb[:], asd1_d[:])

                hT = pa1.tile([HID, NC_NODES], f32)
                asd_own = pa1.tile([2, NC_NODES], f32)
                CK = 448
                for c in range(NC_NODES // CK):
                    sl = slice(c * CK, (c + 1) * CK)
                    ph = pa_ps.tile([HID, CK], f32, tag="ph")
                    nc.tensor.matmul(ph[:], w1a_sb[:], xa_sb[:, sl],
                                     start=True, stop=False)
                    nc.tensor.matmul(ph[:], w1b_sb[:], xb_sb[:, sl],
                                     start=False, stop=True)
                    nc.vector.tensor_copy(hT[:, sl], ph[:])
                    pa2 = pa_ps.tile([2, CK], f32, tag="pa2")
                    nc.tensor.matmul(pa2[:], asd1_sb[:], hT[:, sl],
                                     start=True, stop=True)
                    nc.vector.tensor_copy(asd_own[:, sl], pa2[:])

                for t in range(NTILES):
                    sl = slice(t * 128, (t + 1) * 128)
                    stg = pa.tile([128, 67], fh, tag="stg")
                    pt = pa_ps.tile([128, HID], f32, tag="pt")
                    nc.tensor.transpose(pt[:], hT[:, sl], ident_sb[:HID, :HID])
                    nc.vector.tensor_copy(stg[:, 0:HID], pt[:])
                    nc.vector.memset(stg[:, HID:HID + 1], 1.0)
                    pt2 = pa_ps.tile([128, 2], f32, tag="pt2")
                    nc.tensor.transpose(pt2[:], asd_own[:, sl], ident_sb[:2, :2])
                    nc.vector.tensor_copy(stg[:, HID + 1:HID + 3], pt2[:])
                    nc.sync.dma_start(town[0][sl, 0:67], stg[:])

            # ---------------- per-layer edge pass ----------------
            def edge_layer(layer, ngroups=NPAIR, upto=9):
                if no_ag:
                    pass
                else:
                    nc.gpsimd.collective_compute(
                    "AllGather", OP.bypass,
                    replica_groups=[list(range(CORES))],
                    ins=[town[layer].opt()],
                    outs=[tfull[layer].opt()],
                )
                with (
                    tc.tile_pool(name=f"eg{layer}", bufs=2) as eg,
                    tc.tile_pool(name=f"eb{layer}", bufs=3) as eb,
                    tc.tile_pool(name=f"eps{layer}", bufs=1, space="PSUM") as eps,
                    tc.tile_pool(name=f"ed{layer}", bufs=2) as ed,
                ):
                    for g in range(ngroups):
                        gsl16 = slice(g * GRP // 16, (g + 1) * GRP // 16)
                        gsl128 = slice(g * C, (g + 1) * C)
                        isrc = eg.tile([128, GRP // 16], dt.int16, tag="isrc")
                        idst = eg.tile([128, GRP // 16], dt.int16, tag="idst")
                        nc.sync.dma_start(isrc[:], srcidx_d[:, gsl16])
                        nc.sync.dma_start(idst[:], dstidx_d[:, gsl16])
                        drow = eg.tile([128, C], fh, tag="drow")
                        aesb = eg.tile([128, C], fh, tag="aesb")
                        nc.sync.dma_start(drow[:], dstrow_d[:, gsl128])
                        nc.sync.dma_start(aesb[:], ae_d[layer][:, gsl128])

                        if upto < 1:
                            continue
                        gs = eg.tile([128, C * TBL_W], fh, tag="gs")
                        gs3 = gs[:].rearrange("p (c e) -> p c e", e=TBL_W)
                        gd = eg.tile([128, C * TBL_W], fh, tag="gd")
                        gd3 = gd[:].rearrange("p (c e) -> p c e", e=TBL_W)
                        for j in range(SEGS):
                            csl = slice(j * 2 * RUN_CH, (j + 1) * 2 * RUN_CH)
                            nc.gpsimd.dma_gather(
                                gs3[:, csl, :],
                                tfull[layer][j * SEG_ROWS:(j + 1) * SEG_ROWS, :],
                                isrc[:, j * 2 * run // 16:(j + 1) * 2 * run // 16],
                                2 * run, 2 * run, TBL_W, elem_step=TBL_W,
                                single_packet=False, queue_num=j)
                            if upto >= 2:
                                nc.gpsimd.dma_gather(
                                    gd3[:, csl, :], town[layer][:, :],
                                    idst[:, j * 2 * run // 16:(j + 1) * 2 * run // 16],
                                    2 * run, 2 * run, TBL_W, elem_step=TBL_W,
                                    single_packet=False, queue_num=(j + 2) % 4)

                        if upto < 3:
                            continue
                        # per-edge scalars [128, C]
                        zs = eg.tile([128, C], fh, tag="zs")
                        als = gs3[:, :, HID + 1:HID + 2].rearrange("p c e -> p (c e)")
                        ad = gd3[:, :, HID + 2:HID + 3].rearrange("p c e -> p (c e)")
                        nc.vector.tensor_tensor(zs[:], als, ad, OP.add)
                        nc.vector.tensor_tensor(zs[:], zs[:], aesb[:], OP.add)
                        lkt = eg.tile([128, C], fh, tag="lkt")
                        nc.vector.tensor_scalar(lkt[:], zs[:], NEG_SLOPE, None,
                                                OP.mult)
                        nc.vector.tensor_tensor(lkt[:], lkt[:], zs[:], OP.max)
                        exs = eg.tile([128, C], fh, tag="exs")
                        nc.scalar.activation(exs[:], lkt[:], AF.Exp)

                        if upto < 4:
                            continue
                        pts = [eps.tile([128, HID + 1], f32, tag="acc", bufs=4,
                                        name=f"acc{i}")
                               for i in range(2)]
                        for r in range(8):
                            j, i = r // 2, r % 2
                            rsl = slice(r * RUN_CH, (r + 1) * RUN_CH)
                            bt = eb.tile([128, RUN_CH * 128], fh, tag="bt")
                            bt3 = bt[:].rearrange("p (c e) -> p c e", e=128)
                            nc.vector.tensor_tensor(
                                bt3[:, :, :],
                                iota_sb[:].rearrange("p (q e) -> p q e", q=1)
                                .broadcast_to([128, RUN_CH, 128]),
                                drow[:, rsl].to_broadcast([128, RUN_CH, 128]),
                                OP.is_equal)
                            mex = eb.tile([128, RUN_CH * (HID + 1)], fh, tag="mex")
                            mex3 = mex[:].rearrange("p (c e) -> p c e", e=HID + 1)
                            nc.vector.tensor_tensor(
                                mex3[:, :, :], gs3[:, rsl, 0:HID + 1],
                                exs[:, rsl].to_broadcast([128, RUN_CH, HID + 1]),
                                OP.mult)
                            for cc in range(RUN_CH):
                                nc.tensor.matmul(
                                    pts[i][:], bt3[:, cc, :], mex3[:, cc, :],
                                    start=(j == 0 and cc == 0),
                                    stop=(j == 3 and cc == RUN_CH - 1))

                        if upto < 5:
                            continue
                        for i in range(2):
                            t = 2 * g + i
                            sl = slice(t * 128, (t + 1) * 128)
                            dsb = ed.tile([128, 1], f32, tag="dsb")
                            nc.vector.tensor_scalar(
                                dsb[:], pts[i][:, HID:HID + 1], 1e-16, None,
                                OP.add)
                            if layer == 0:
                                stg = ed.tile([128, 67], fh, tag="stg2")
                                tb = ed.tile([128, HID], f32, tag="tb")
                                nc.vector.tensor_scalar(
                                    tb[:], b1rep_sb[:], dsb[:], None, OP.mult)
                                nc.vector.tensor_tensor(
                                    tb[:], tb[:], pts[i][:, 0:HID], OP.add)
                                nc.scalar.activation(tb[:], tb[:], AF.Relu)
                                inv = ed.tile([128, 1], f32, tag="inv")
                                nc.vector.reciprocal(inv[:], dsb[:])
                                nc.vector.tensor_scalar(
                                    stg[:, 0:HID], tb[:], inv[:], None,
                                    OP.mult)
                                nc.vector.memset(stg[:, HID:HID + 1], 1.0)
                                tmp = ed.tile([128, HID], fh, tag="tmp")
                                red = ed.tile([128, 2], f32, tag="red")
                                nc.vector.tensor_tensor(
                                    tmp[:], stg[:, 0:HID], avs2_sb[:], OP.mult)
                                nc.vector.reduce_sum(
                                    red[:, 0:1], tmp[:], axis=AX.X)
                                nc.vector.tensor_tensor(
                                    tmp[:], stg[:, 0:HID], avd2_sb[:], OP.mult)
                                nc.vector.reduce_sum(
                                    red[:, 1:2], tmp[:], axis=AX.X)
                                nc.vector.tensor_copy(
                                    stg[:, HID + 1:HID + 3], red[:])
                                nc.sync.dma_start(town[1][sl, 0:67], stg[:])
                            else:
                                msb = ed.tile([128, HID], f32, tag="msb")
                                inv = ed.tile([128, 1], f32, tag="inv")
                                nc.vector.reciprocal(inv[:], dsb[:])
                                nc.vector.tensor_scalar(
                                    msb[:], pts[i][:, 0:HID], inv[:], None,
                                    OP.mult)
                                pmT = eps.tile([HID, 128], f32, tag="pmT", bufs=2)
                                nc.tensor.transpose(pmT[:], msb[:], ident_sb[:])
                                mT = ed.tile([HID, 128], f32, tag="mT")
                                nc.vector.tensor_copy(mT[:], pmT[:])
                                ph2 = eps.tile([HID, 128], f32, tag="ph2", bufs=1)
                                nc.tensor.matmul(ph2[:], w2_sb[:], mT[:],
                                                 start=True, stop=True)
                                h2T = ed.tile([HID, 128], f32, tag="h2T")
                                nc.scalar.activation(h2T[:], ph2[:], AF.Relu,
                                                     bias=b2col_sb[:])
                                po = eps.tile([2, 128], f32, tag="po", bufs=1)
                                nc.tensor.matmul(po[:], wlina_sb[:], h2T[:],
                                                 start=True, stop=False)
                                nc.tensor.matmul(po[:], wlinb_sb[:],
                                                 xb_sb[:, sl],
                                                 start=False, stop=True)
                                oT = ed.tile([2, 128], f32, tag="oT")
                                nc.scalar.activation(oT[:], po[:], AF.Relu,
                                                     bias=blin_sb[:])
                                nc.sync.dma_start(out_d[:, sl], oT[:])

            if stage == "phasea":
                with tc.tile_pool(name="dbgp", bufs=2) as dbgp:
                    for t in range(NTILES):
                        sl = slice(t * 128, (t + 1) * 128)
                        dd = dbgp.tile([128, TBL_W], f32, tag="dd")
                        nc.sync.dma_start(dd[:], town[0][sl, :])
                        nc.sync.dma_start(dbg_d[sl, :], dd[:])
            elif stage == "ag":
                nc.gpsimd.collective_compute(
                    "AllGather", OP.bypass,
                    replica_groups=[list(range(CORES))],
                    ins=[town[0].opt()],
                    outs=[tfull[0].opt()],
                )
                with tc.tile_pool(name="dbgp", bufs=2) as dbgp:
                    for t in range(NTILES):
                        sl = slice(t * 128, (t + 1) * 128)
                        dd = dbgp.tile([128, TBL_W], f32, tag="dd")
                        nc.sync.dma_start(dd[:], tfull[0][sl, :])
                        nc.sync.dma_start(dbg_d[sl, :], dd[:])
            elif stage == "layer1":
                edge_layer(0, ngroups, upto)
            elif stage == "l1dump":
                edge_layer(0, ngroups, upto)
                with tc.tile_pool(name="dbgp", bufs=2) as dbgp:
                    for t in range(NTILES):
                        sl = slice(t * 128, (t + 1) * 128)
                        dd = dbgp.tile([128, TBL_W], f32, tag="dd")
                        nc.sync.dma_start(dd[:], town[1][sl, :])
                        nc.sync.dma_start(dbg_d[sl, :], dd[:])
            else:
                edge_layer(0, ngroups, upto)
                edge_layer(1, ngroups, upto)

    nc.compile()
    return nc


def _get_program(run):
    if run not in _PROG_CACHE:
        _PROG_CACHE[run] = _build_program(run)
    return _PROG_CACHE[run]


def kernel(**inputs):
    from concourse.bass_utils import run_bass_kernel_spmd

    per_core, consts, run = _prep(**inputs)
    nc = _get_program(run)
    in_maps = [dict(consts, **pc) for pc in per_core]
    res = run_bass_kernel_spmd(nc, in_maps, core_ids=list(range(CORES)))
    out = np.concatenate([r["out"].T for r in res.results], axis=0)[:N]
    return np.ascontiguousarray(out)



# revision 23
# speedup vs baseline: 1.1685x; 1.1361x over previous
"""EnhancedGATCN Trainium2 kernel: 2-layer GAT (heads=1, edge attrs) + linear head.

Strategy (8 NeuronCores, SPMD):
  - Destination-node sharding: core k owns dst nodes [k*12544, (k+1)*12544).
  - Per layer: a 256B/row fp16 gather table T[n] = [h[n](64) | 1 |
    alpha_src[n] | alpha_dst[n] | pad] is AllGathered across cores; each
    core dma_gathers T[src[e]] from the global table (int16 idx => 4 table
    segments of 25088 rows) and T_own[dst[e]] from its local shard (for
    alpha_dst, col 66), computes edge softmax numerators ex[e] =
    exp(leaky_relu(as+ad+ae)) in fp16, and aggregates per dst-tile with a
    one-hot matmul: psum[128 dst, 65] += B_t (lhsT, fp16) @ (ex*[h|1]).
  - Segment max is skipped (logits are O(10); exp is safe and the max
    cancels exactly in the softmax ratio).
  - W2 is applied after aggregation (linearity); the final linear layer is
    fused into the layer-2 drain in transposed layout (all f32).

Host-side prep (numpy): shard + sort edges by (dst tile pair, src segment),
pad each (tile,seg) run to a fixed RUN slots so all 8 cores share one
instruction stream (SPMD), build int16 gather indices and fp16 slot arrays.

alpha_dst handling (no dst gather stream at all):
  - Layer 1: alpha_dst1 = (x||x_ext)@W1@att_dst1 depends only on inputs,
    so the host folds it into the ae1 edge array (exact, f32 on host).
  - Layer 2: alpha_dst2 of the pair's 256 candidate dst nodes is loaded
    [1,256] from a td1 column written by the layer-1 drain,
    partition_broadcast to [128,256], and selected per-slot with the SAME
    one-hot tiles the aggregation needs anyway (mult+reduce on DVE).

Measured (8 axon trn2 cores): Relative L2 error 1.08e-4 vs the f32 jax
reference (fp16 table/pipeline; F16=False is ~4.7e-7, ~1ms slower).
The wall-clock of one execute is dominated by a ~71-89 ms axon/PJRT
dispatch floor (a null program costs that much; it also drifts several ms
over tens of minutes, so only same-process A/B comparisons are valid).
Device-attributable time is ~3-4 ms: src dma_gather stream (451k rows/
core/layer x 256B; ~8 ns/row on one SWDGE queue, ~2.2 ns/row spread over
4 queues via num_swdge_queues=4 + queue_num; nearly fully overlapped),
DVE one-hot/alpha_dst2/mex ~1.4 ms, drain ~1.1 ms, AllGathers hidden.
A dst-side dma_gather stream (alpha_dst from the local shard) was
measured at +3.2 ms exposed and removed in favor of the above.
Note dma_gather requires single_packet=False above 1024 idxs per call,
256B-multiple rows, int16 indices (hence the 4-segment table split), and
the idx tile replicated across the 8 Q7 cores ([128, n/16] = 8 copies of
[16, n/16]); partition_broadcast requires in_ap at base partition 0.
"""

import numpy as np

F16 = True  # use fp16 table/one-hot/scalar pipeline

# ---- problem constants (hardcoded per task instructions) ----
N = 100_000
E = 3_200_000
IN_CH, HID, EXT = 128, 64, 3
NEG_SLOPE = 0.2
CORES = 8
NC_NODES = 12544            # 98 tiles of 128 per core
NTILES = NC_NODES // 128    # 98
NPAD = CORES * NC_NODES     # 100352
SEGS = 4
SEG_ROWS = NPAD // SEGS     # 25088  (< 32768 so int16 indices fit)
NPAIR = NTILES // 2         # 49 tile pairs per core
TBL_W = 128                 # table row width: 128 fp16 -> 256B rows


def _prep(x, x_ext, edge_index, edge_weight,
          W1, att_src1, att_dst1, We1, att_e1, b1,
          W2, att_src2, att_dst2, We2, att_e2, b2,
          Wlin, blin):
    """Host prep. Returns (per_core_inputs: list[dict], consts: dict, run)."""
    x = np.asarray(x, np.float32)
    x_ext = np.asarray(x_ext, np.float32)
    src = np.asarray(edge_index[0], np.int64)
    dst = np.asarray(edge_index[1], np.int64)
    w = np.asarray(edge_weight, np.float32).reshape(-1)

    k1 = float(np.asarray(We1, np.float32).reshape(-1) @ np.asarray(att_e1, np.float32))
    k2 = float(np.asarray(We2, np.float32).reshape(-1) @ np.asarray(att_e2, np.float32))
    # layer-1 alpha_dst depends only on inputs: fold it into the ae1 stream
    xcat_h = np.concatenate([np.asarray(x, np.float32),
                             np.asarray(x_ext, np.float32)], axis=1)
    ad1_full = (xcat_h @ np.asarray(W1, np.float32))         @ np.asarray(att_dst1, np.float32)             # [N]

    core = dst // NC_NODES
    tile_l = (dst % NC_NODES) // 128        # 0..97
    seg = src // SEG_ROWS                   # 0..3
    # group index within core: (tile//2)*8 + seg*2 + (tile&1)
    grp = (tile_l // 2) * 8 + seg * 2 + (tile_l & 1)
    NGRP = NPAIR * 8  # 392

    # fixed RUN per (tile, seg) across all cores
    flat = core * NGRP + grp
    counts = np.bincount(flat, minlength=CORES * NGRP)
    run = int(np.ceil(counts.max() / 128.0) * 128)
    S = NGRP * run

    xcat = np.concatenate([x, x_ext], axis=1)          # [N, 131]
    xcat_pad = np.zeros((NPAD, IN_CH + EXT), np.float32)
    xcat_pad[:N] = xcat

    order = np.argsort(flat, kind="stable")
    fs, base = flat[order], None
    # rank within group
    cum = np.zeros(CORES * NGRP + 1, np.int64)
    np.cumsum(counts, out=cum[1:])
    rank = np.arange(E, dtype=np.int64) - cum[fs]
    slot = (fs % NGRP) * run + rank                    # slot within its core
    core_o = fs // NGRP

    per_core = []
    for k in range(CORES):
        m = core_o == k
        sl = slot[m]
        e = order[m]
        src16 = np.zeros(S, np.int16)
        dst16 = np.zeros(S, np.int16)
        dstrow = np.full(S, -1.0, np.float32)
        ae1 = np.zeros(S, np.float32)
        ae2 = np.zeros(S, np.float32)
        src16[sl] = (src[e] - seg[e] * SEG_ROWS).astype(np.int16)
        dst16[sl] = (dst[e] % NC_NODES).astype(np.int16)
        dstrow[sl] = (dst[e] % 128).astype(np.float32)
        ae1[sl] = k1 * w[e] + ad1_full[dst[e]]
        ae2[sl] = k2 * w[e]

        def wrap16(a):  # [128, S/16] replicated for the 8 Q7 cores
            return np.tile(a.reshape(S // 16, 16).T, (8, 1)).copy()

        def wrap128(a):  # [128, S/128], slot s -> [s%128, s//128]
            return a.reshape(S // 128, 128).T.copy()

        xcT = xcat_pad[k * NC_NODES:(k + 1) * NC_NODES].T.copy()  # [131, NC]
        hdt = np.float16 if F16 else np.float32
        per_core.append({
            "srcidx": wrap16(src16),
            "dstidx": wrap16(dst16),
            "dstrow": wrap128(dstrow).astype(hdt),
            "ae1": wrap128(ae1).astype(hdt),
            "ae2": wrap128(ae2).astype(hdt),
            "xa": np.ascontiguousarray(xcT[:IN_CH]),       # [128, NC]
            "xb": np.ascontiguousarray(xcT[IN_CH:]),       # [3, NC]
        })

    W1 = np.asarray(W1, np.float32)
    W2 = np.asarray(W2, np.float32)
    Wlin = np.asarray(Wlin, np.float32)
    consts = {
        "w1a": np.ascontiguousarray(W1[:IN_CH]),           # [128, 64]
        "w1b": np.ascontiguousarray(W1[IN_CH:]),           # [3, 64]
        "asd1": np.stack([np.asarray(att_src1, np.float32),
                          np.asarray(att_dst1, np.float32)], 1),  # [64, 2]
        "w2": W2,                                          # [64, 64] (lhsT)
        "avs2": np.tile(W2 @ np.asarray(att_src2, np.float32),
                        (128, 1)).astype(np.float16 if F16 else np.float32),
        "avd2": np.tile(W2 @ np.asarray(att_dst2, np.float32),
                        (128, 1)).astype(np.float16 if F16 else np.float32),
        "b1rep": np.tile(np.asarray(b1, np.float32), (128, 1)),   # [128, 64]
        "b2col": np.asarray(b2, np.float32).reshape(HID, 1),      # [64, 1]
        "wlina": np.ascontiguousarray(Wlin[:HID]),         # [64, 2]
        "wlinb": np.ascontiguousarray(Wlin[HID:]),         # [3, 2]
        "blincol": np.asarray(blin, np.float32).reshape(2, 1),    # [2, 1]
        "iota": np.tile(np.arange(128, dtype=np.float16 if F16 else np.float32),
                        (128, 1)),  # [128,128]
        "ident": np.eye(128, dtype=np.float32),
    }
    return per_core, consts, run


def _mirror(per_core, consts, run):
    """Numpy mirror of the device algorithm (for validation)."""
    S = 392 * run
    outs = []
    tfull = [None, None]  # layer tables
    # ---- phase A on every core: build T1 shard ----
    shards = []
    td1 = []
    for k in range(CORES):
        pc = per_core[k]
        xc = np.concatenate([pc["xa"], pc["xb"]], 0).T  # [NC, 131]
        W1 = np.concatenate([consts["w1a"], consts["w1b"]], 0)
        h = xc @ W1                                    # [NC, 64]
        asd = h @ consts["asd1"]                       # [NC, 2]
        T = np.zeros((NC_NODES, TBL_W), np.float32)
        T[:, :HID] = h
        T[:, HID] = 1.0
        T[:, HID + 1] = asd[:, 0]
        T[:, HID + 2] = asd[:, 1]
        shards.append(T)
        td1.append(asd[:, 1].copy())
    tfull[0] = np.concatenate(shards, 0)               # [NPAD, 128]

    def unwrap16(a):  # inverse of wrap16
        return a[:16].T.reshape(-1)

    def unwrap128(a):
        return a.T.reshape(-1)

    def edge_pass(k, tbl, td, ae_key, layer):
        pc = per_core[k]
        src16 = unwrap16(pc["srcidx"]).astype(np.int64)
        dstrow = unwrap128(pc["dstrow"])
        ae = unwrap128(pc[ae_key])
        num = np.zeros((NC_NODES, HID + 1), np.float32)
        # segment base per slot
        segid = (np.arange(S) // run) % 8 // 2
        gsrc = tbl[src16 + segid * SEG_ROWS]           # [S, 128]
        tof = (np.arange(S) // (8 * run)) * 2 + (np.arange(S) // run) % 2
        d16 = tof * 128 + np.maximum(dstrow, 0).astype(np.int64)
        ad = np.where(dstrow >= 0, td[d16], 0.0)       # [S]
        als = gsrc[:, HID + 1]
        z = als + ad + ae
        l = np.where(z > 0, z, NEG_SLOPE * z).astype(np.float32)
        ex = np.exp(l).astype(np.float32)
        mex = gsrc[:, :HID + 1] * ex[:, None]
        tile_of_slot = (np.arange(S) // (8 * run)) * 2 + (np.arange(S) // run) % 2
        valid = dstrow >= 0
        d_global = tile_of_slot * 128 + dstrow.astype(np.int64)
        np.add.at(num, d_global[valid], mex[valid])
        return num

    # ---- layer 1 ----
    shards2 = []
    td2 = []
    for k in range(CORES):
        num = edge_pass(k, tfull[0], td1[k], "ae1", 1)
        denp = num[:, HID] + 1e-16
        h1 = np.maximum(num[:, :HID] + consts["b1rep"][0][None, :] * denp[:, None], 0) / denp[:, None]
        a2s = h1 @ consts["avs2"][0]
        a2d = h1 @ consts["avd2"][0]
        T = np.zeros((NC_NODES, TBL_W), np.float32)
        T[:, :HID] = h1
        T[:, HID] = 1.0
        T[:, HID + 1] = a2s
        T[:, HID + 2] = a2d
        shards2.append(T)
        td2.append(a2d.copy())
    tfull[1] = np.concatenate(shards2, 0)

    # ---- layer 2 + final ----
    for k in range(CORES):
        num = edge_pass(k, tfull[1], td2[k], "ae2", 2)
        denp = num[:, HID] + 1e-16
        m = num[:, :HID] / denp[:, None]
        h2 = np.maximum(m @ consts["w2"] + consts["b2col"][:, 0][None, :], 0)
        xe = per_core[k]["xb"].T                        # [NC, 3]
        o = np.maximum(h2 @ consts["wlina"] + xe @ consts["wlinb"]
                       + consts["blincol"][:, 0][None, :], 0)
        outs.append(o)
    return np.concatenate(outs, 0)[:N]


# ============================ BASS PROGRAM ============================

_PROG_CACHE = {}


def _build_program(run, stage="full", ngroups=NPAIR, upto=9, n_devices=CORES,
                   no_ag=False):
    import concourse.bacc as bacc
    import concourse.mybir as mybir
    import concourse.tile as tile
    dt = mybir.dt
    f32 = dt.float32
    f16 = dt.float16
    fh = f16 if F16 else f32

    RUN_CH = run // 128
    C = 8 * RUN_CH            # chunks per pair-group
    GRP = 8 * run             # slots per pair-group
    S = NPAIR * GRP

    nc = bacc.Bacc("TRN2", target_bir_lowering=False, debug=False,
                   num_devices=n_devices, num_swdge_queues=4)

    def din(name, shape, d=f32):
        return nc.dram_tensor(name, shape, d, kind="ExternalInput")

    srcidx_d = din("srcidx", [128, S // 16], dt.int16)
    dstidx_d = din("dstidx", [128, S // 16], dt.int16)
    dstrow_d = din("dstrow", [128, S // 128], fh)
    ae_d = [din("ae1", [128, S // 128], fh),
            din("ae2", [128, S // 128], fh)]
    xa_d = din("xa", [128, NC_NODES])
    xb_d = din("xb", [3, NC_NODES])
    w1a_d = din("w1a", [IN_CH, HID])
    w1b_d = din("w1b", [3, HID])
    asd1_d = din("asd1", [HID, 2])
    w2_d = din("w2", [HID, HID])
    avs2_d = din("avs2", [128, HID], fh)
    avd2_d = din("avd2", [128, HID], fh)
    b1rep_d = din("b1rep", [128, HID])
    b2col_d = din("b2col", [HID, 1])
    wlina_d = din("wlina", [HID, 2])
    wlinb_d = din("wlinb", [3, 2])
    blin_d = din("blincol", [2, 1])
    iota_d = din("iota", [128, 128], fh)
    ident_d = din("ident", [128, 128])
    out_d = nc.dram_tensor("out", [2, NC_NODES], f32, kind="ExternalOutput")
    dbg_d = None
    if stage in ("phasea", "ag", "l1dump"):
        dbg_d = nc.dram_tensor("dbg", [NC_NODES, TBL_W], f32,
                               kind="ExternalOutput")

    AX = mybir.AxisListType
    OP = mybir.AluOpType
    AF = mybir.ActivationFunctionType

    with tile.TileContext(nc) as tc:
        with (
            tc.tile_pool(name="dram", bufs=1, space="DRAM") as dram,
            tc.tile_pool(name="const", bufs=1) as cpool,
            tc.tile_pool(name="persist", bufs=1) as ppool,
        ):
            town = [dram.tile([NC_NODES, TBL_W], fh, name=f"town{i}")
                    for i in range(2)]
            td1 = dram.tile([NC_NODES, 1], fh, name="td1")
            tfull = [dram.tile([NPAD, TBL_W], fh, name=f"tfull{i}",
                              addr_space="Shared")
                     for i in range(2)]

            iota_sb = cpool.tile([128, 128], fh)
            ident_sb = cpool.tile([128, 128], f32)
            w2_sb = cpool.tile([HID, HID], f32)
            avs2_sb = cpool.tile([128, HID], fh)
            avd2_sb = cpool.tile([128, HID], fh)
            b1rep_sb = cpool.tile([128, HID], f32)
            b2col_sb = cpool.tile([HID, 1], f32)
            wlina_sb = cpool.tile([HID, 2], f32)
            wlinb_sb = cpool.tile([3, 2], f32)
            blin_sb = cpool.tile([2, 1], f32)
            xb_sb = ppool.tile([3, NC_NODES], f32)
            for sb, d in [(iota_sb, iota_d), (ident_sb, ident_d),
                          (w2_sb, w2_d), (avs2_sb, avs2_d), (avd2_sb, avd2_d),
                          (b1rep_sb, b1rep_d), (b2col_sb, b2col_d),
                          (wlina_sb, wlina_d), (wlinb_sb, wlinb_d),
                          (blin_sb, blin_d), (xb_sb, xb_d)]:
                nc.sync.dma_start(sb[:], d[:])

            # ---------------- phase A: h1 table ----------------
            with (
                tc.tile_pool(name="pa", bufs=2) as pa,
                tc.tile_pool(name="pa_ps", bufs=2, space="PSUM") as pa_ps,
                tc.tile_pool(name="pa1", bufs=1) as pa1,
            ):
                xa_sb = pa1.tile([128, NC_NODES], f32)
                nc.sync.dma_start(xa_sb[:], xa_d[:])
                w1a_sb = pa1.tile([IN_CH, HID], f32)
                w1b_sb = pa1.tile([3, HID], f32)
                asd1_sb = pa1.tile([HID, 2], f32)
                nc.sync.dma_start(w1a_sb[:], w1a_d[:])
                nc.sync.dma_start(w1b_sb[:], w1b_d[:])
                nc.sync.dma_start(asd1_sb[:], asd1_d[:])

                hT = pa1.tile([HID, NC_NODES], f32)
                asd_own = pa1.tile([2, NC_NODES], f32)
                CK = 448
                for c in range(NC_NODES // CK):
                    sl = slice(c * CK, (c + 1) * CK)
                    ph = pa_ps.tile([HID, CK], f32, tag="ph")
                    nc.tensor.matmul(ph[:], w1a_sb[:], xa_sb[:, sl],
                                     start=True, stop=False)
                    nc.tensor.matmul(ph[:], w1b_sb[:], xb_sb[:, sl],
                                     start=False, stop=True)
                    nc.vector.tensor_copy(hT[:, sl], ph[:])
                    pa2 = pa_ps.tile([2, CK], f32, tag="pa2")
                    nc.tensor.matmul(pa2[:], asd1_sb[:], hT[:, sl],
                                     start=True, stop=True)
                    nc.vector.tensor_copy(asd_own[:, sl], pa2[:])

                for t in range(NTILES):
                    sl = slice(t * 128, (t + 1) * 128)
                    stg = pa.tile([128, 67], fh, tag="stg")
                    pt = pa_ps.tile([128, HID], f32, tag="pt")
                    nc.tensor.transpose(pt[:], hT[:, sl], ident_sb[:HID, :HID])
                    nc.vector.tensor_copy(stg[:, 0:HID], pt[:])
                    nc.vector.memset(stg[:, HID:HID + 1], 1.0)
                    pt2 = pa_ps.tile([128, 2], f32, tag="pt2")
                    nc.tensor.transpose(pt2[:], asd_own[:, sl], ident_sb[:2, :2])
                    nc.vector.tensor_copy(stg[:, HID + 1:HID + 3], pt2[:])
                    nc.sync.dma_start(town[0][sl, 0:67], stg[:])

            # ---------------- per-layer edge pass ----------------
            def edge_layer(layer, ngroups=NPAIR, upto=9):
                if no_ag:
                    pass
                else:
                    nc.gpsimd.collective_compute(
                    "AllGather", OP.bypass,
                    replica_groups=[list(range(CORES))],
                    ins=[town[layer].opt()],
                    outs=[tfull[layer].opt()],
                )
                with (
                    tc.tile_pool(name=f"eg{layer}", bufs=2) as eg,
                    tc.tile_pool(name=f"eb{layer}", bufs=3) as eb,
                    tc.tile_pool(name=f"eps{layer}", bufs=1, space="PSUM") as eps,
                    tc.tile_pool(name=f"ed{layer}", bufs=2) as ed,
                ):
                    for g in range(ngroups):
                        gsl16 = slice(g * GRP // 16, (g + 1) * GRP // 16)
                        gsl128 = slice(g * C, (g + 1) * C)
                        isrc = eg.tile([128, GRP // 16], dt.int16, tag="isrc")
                        nc.sync.dma_start(isrc[:], srcidx_d[:, gsl16])
                        drow = eg.tile([128, C], fh, tag="drow")
                        aesb = eg.tile([128, C], fh, tag="aesb")
                        nc.sync.dma_start(drow[:], dstrow_d[:, gsl128])
                        nc.sync.dma_start(aesb[:], ae_d[layer][:, gsl128])

                        if upto < 1:
                            continue
                        gs = eg.tile([128, C * TBL_W], fh, tag="gs")
                        gs3 = gs[:].rearrange("p (c e) -> p c e", e=TBL_W)
                        for j in range(SEGS):
                            csl = slice(j * 2 * RUN_CH, (j + 1) * 2 * RUN_CH)
                            nc.gpsimd.dma_gather(
                                gs3[:, csl, :],
                                tfull[layer][j * SEG_ROWS:(j + 1) * SEG_ROWS, :],
                                isrc[:, j * 2 * run // 16:(j + 1) * 2 * run // 16],
                                2 * run, 2 * run, TBL_W, elem_step=TBL_W,
                                single_packet=False, queue_num=j)

                        if upto < 3:
                            continue
                        # per-edge scalars [128, C]
                        bts = []
                        if layer == 1:
                            tdl = eg.tile([1, 256], fh, tag="tdl")
                            nc.sync.dma_start(
                                tdl[:],
                                td1[g * 256:(g + 1) * 256, 0:1]
                                .rearrange("(o e) one -> o (e one)", o=1))
                            tdr = eg.tile([128, 256], fh, tag="tdr")
                            nc.gpsimd.partition_broadcast(tdr[:], tdl[:])
                            adc = eg.tile([128, C], fh, tag="adc")
                            adc3 = adc[:].rearrange("p (c e) -> p c e", e=1)
                            for r in range(8):
                                i = r % 2
                                rsl = slice(r * RUN_CH, (r + 1) * RUN_CH)
                                bt = eb.tile([128, RUN_CH * 128], fh, tag="bt",
                                             bufs=9)
                                bt3 = bt[:].rearrange("p (c e) -> p c e", e=128)
                                nc.vector.tensor_tensor(
                                    bt3[:, :, :],
                                    iota_sb[:].rearrange("p (q e) -> p q e",
                                                         q=1)
                                    .broadcast_to([128, RUN_CH, 128]),
                                    drow[:, rsl].to_broadcast(
                                        [128, RUN_CH, 128]),
                                    OP.is_equal)
                                bts.append(bt3)
                                adt = eb.tile([128, RUN_CH * 128], fh,
                                              tag="adt", bufs=2)
                                adt3 = adt[:].rearrange("p (c e) -> p c e",
                                                        e=128)
                                nc.vector.tensor_tensor(
                                    adt3[:, :, :], bt3[:, :, :],
                                    tdr[:, i * 128:(i + 1) * 128]
                                    .rearrange("p (q e) -> p q e", q=1)
                                    .broadcast_to([128, RUN_CH, 128]),
                                    OP.mult)
                                with nc.allow_low_precision(
                                        "one-hot row: <=1 nonzero, exact"):
                                    nc.vector.reduce_sum(adc3[:, rsl, :],
                                                         adt3[:, :, :],
                                                         axis=AX.X)
                        zs = eg.tile([128, C], fh, tag="zs")
                        als = gs3[:, :, HID + 1:HID + 2].rearrange("p c e -> p (c e)")
                        if layer == 1:
                            nc.vector.tensor_tensor(zs[:], als, adc[:], OP.add)
                            nc.vector.tensor_tensor(zs[:], zs[:], aesb[:],
                                                    OP.add)
                        else:
                            nc.vector.tensor_tensor(zs[:], als, aesb[:],
                                                    OP.add)
                        lkt = eg.tile([128, C], fh, tag="lkt")
                        nc.vector.tensor_scalar(lkt[:], zs[:], NEG_SLOPE, None,
                                                OP.mult)
                        nc.vector.tensor_tensor(lkt[:], lkt[:], zs[:], OP.max)
                        exs = eg.tile([128, C], fh, tag="exs")
                        nc.scalar.activation(exs[:], lkt[:], AF.Exp)

                        if upto < 4:
                            continue
                        pts = [eps.tile([128, HID + 1], f32, tag="acc", bufs=4,
                                        name=f"acc{i}")
                               for i in range(2)]
                        for r in range(8):
                            j, i = r // 2, r % 2
                            rsl = slice(r * RUN_CH, (r + 1) * RUN_CH)
                            if layer == 1:
                                bt3 = bts[r]
                            else:
                                bt = eb.tile([128, RUN_CH * 128], fh,
                                             tag="bt", bufs=9)
                                bt3 = bt[:].rearrange("p (c e) -> p c e",
                                                      e=128)
                                nc.vector.tensor_tensor(
                                    bt3[:, :, :],
                                    iota_sb[:].rearrange("p (q e) -> p q e",
                                                         q=1)
                                    .broadcast_to([128, RUN_CH, 128]),
                                    drow[:, rsl].to_broadcast(
                                        [128, RUN_CH, 128]),
                                    OP.is_equal)
                            mex = eb.tile([128, RUN_CH * (HID + 1)], fh, tag="mex")
                            mex3 = mex[:].rearrange("p (c e) -> p c e", e=HID + 1)
                            nc.vector.tensor_tensor(
                                mex3[:, :, :], gs3[:, rsl, 0:HID + 1],
                                exs[:, rsl].to_broadcast([128, RUN_CH, HID + 1]),
                                OP.mult)
                            for cc in range(RUN_CH):
                                nc.tensor.matmul(
                                    pts[i][:], bt3[:, cc, :], mex3[:, cc, :],
                                    start=(j == 0 and cc == 0),
                                    stop=(j == 3 and cc == RUN_CH - 1))

                        if upto < 5:
                            continue
                        for i in range(2):
                            t = 2 * g + i
                            sl = slice(t * 128, (t + 1) * 128)
                            dsb = ed.tile([128, 1], f32, tag="dsb")
                            nc.vector.tensor_scalar(
                                dsb[:], pts[i][:, HID:HID + 1], 1e-16, None,
                                OP.add)
                            if layer == 0:
                                stg = ed.tile([128, 67], fh, tag="stg2")
                                tb = ed.tile([128, HID], f32, tag="tb")
                                nc.vector.tensor_scalar(
                                    tb[:], b1rep_sb[:], dsb[:], None, OP.mult)
                                nc.vector.tensor_tensor(
                                    tb[:], tb[:], pts[i][:, 0:HID], OP.add)
                                nc.scalar.activation(tb[:], tb[:], AF.Relu)
                                inv = ed.tile([128, 1], f32, tag="inv")
                                nc.vector.reciprocal(inv[:], dsb[:])
                                nc.vector.tensor_scalar(
                                    stg[:, 0:HID], tb[:], inv[:], None,
                                    OP.mult)
                                nc.vector.memset(stg[:, HID:HID + 1], 1.0)
                                tmp = ed.tile([128, HID], fh, tag="tmp")
                                red = ed.tile([128, 2], f32, tag="red")
                                nc.vector.tensor_tensor(
                                    tmp[:], stg[:, 0:HID], avs2_sb[:], OP.mult)
                                nc.vector.reduce_sum(
                                    red[:, 0:1], tmp[:], axis=AX.X)
                                nc.vector.tensor_tensor(
                                    tmp[:], stg[:, 0:HID], avd2_sb[:], OP.mult)
                                nc.vector.reduce_sum(
                                    red[:, 1:2], tmp[:], axis=AX.X)
                                nc.vector.tensor_copy(
                                    stg[:, HID + 1:HID + 3], red[:])
                                nc.sync.dma_start(town[1][sl, 0:67], stg[:])
                                nc.sync.dma_start(td1[sl, 0:1],
                                                  stg[:, HID + 2:HID + 3])
                            else:
                                msb = ed.tile([128, HID], f32, tag="msb")
                                inv = ed.tile([128, 1], f32, tag="inv")
                                nc.vector.reciprocal(inv[:], dsb[:])
                                nc.vector.tensor_scalar(
                                    msb[:], pts[i][:, 0:HID], inv[:], None,
                                    OP.mult)
                                pmT = eps.tile([HID, 128], f32, tag="pmT", bufs=2)
                                nc.tensor.transpose(pmT[:], msb[:], ident_sb[:])
                                mT = ed.tile([HID, 128], f32, tag="mT")
                                nc.vector.tensor_copy(mT[:], pmT[:])
                                ph2 = eps.tile([HID, 128], f32, tag="ph2", bufs=1)
                                nc.tensor.matmul(ph2[:], w2_sb[:], mT[:],
                                                 start=True, stop=True)
                                h2T = ed.tile([HID, 128], f32, tag="h2T")
                                nc.scalar.activation(h2T[:], ph2[:], AF.Relu,
                                                     bias=b2col_sb[:])
                                po = eps.tile([2, 128], f32, tag="po", bufs=1)
                                nc.tensor.matmul(po[:], wlina_sb[:], h2T[:],
                                                 start=True, stop=False)
                                nc.tensor.matmul(po[:], wlinb_sb[:],
                                                 xb_sb[:, sl],
                                                 start=False, stop=True)
                                oT = ed.tile([2, 128], f32, tag="oT")
                                nc.scalar.activation(oT[:], po[:], AF.Relu,
                                                     bias=blin_sb[:])
                                nc.sync.dma_start(out_d[:, sl], oT[:])

            if stage == "phasea":
                with tc.tile_pool(name="dbgp", bufs=2) as dbgp:
                    for t in range(NTILES):
                        sl = slice(t * 128, (t + 1) * 128)
                        dd = dbgp.tile([128, TBL_W], f32, tag="dd")
                        nc.sync.dma_start(dd[:], town[0][sl, :])
                        nc.sync.dma_start(dbg_d[sl, :], dd[:])
            elif stage == "ag":
                nc.gpsimd.collective_compute(
                    "AllGather", OP.bypass,
                    replica_groups=[list(range(CORES))],
                    ins=[town[0].opt()],
                    outs=[tfull[0].opt()],
                )
                with tc.tile_pool(name="dbgp", bufs=2) as dbgp:
                    for t in range(NTILES):
                        sl = slice(t * 128, (t + 1) * 128)
                        dd = dbgp.tile([128, TBL_W], f32, tag="dd")
                        nc.sync.dma_start(dd[:], tfull[0][sl, :])
                        nc.sync.dma_start(dbg_d[sl, :], dd[:])
            elif stage == "layer1":
                edge_layer(0, ngroups, upto)
            elif stage == "l1dump":
                edge_layer(0, ngroups, upto)
                with tc.tile_pool(name="dbgp", bufs=2) as dbgp:
                    for t in range(NTILES):
                        sl = slice(t * 128, (t + 1) * 128)
                        dd = dbgp.tile([128, TBL_W], f32, tag="dd")
                        nc.sync.dma_start(dd[:], town[1][sl, :])
                        nc.sync.dma_start(dbg_d[sl, :], dd[:])
            else:
                edge_layer(0, ngroups, upto)
                edge_layer(1, ngroups, upto)

    nc.compile()
    return nc


def _get_program(run):
    if run not in _PROG_CACHE:
        _PROG_CACHE[run] = _build_program(run)
    return _PROG_CACHE[run]


def kernel(**inputs):
    from concourse.bass_utils import run_bass_kernel_spmd

    per_core, consts, run = _prep(**inputs)
    nc = _get_program(run)
    in_maps = [dict(consts, **pc) for pc in per_core]
    res = run_bass_kernel_spmd(nc, in_maps, core_ids=list(range(CORES)))
    out = np.concatenate([r["out"].T for r in res.results], axis=0)[:N]
    return np.ascontiguousarray(out)

